# revision 68
# baseline (speedup 1.0000x reference)
"""Multi-head attention (B=2, S=2048, D=1024, H=16, d_k=64) on 8 NeuronCores.

Sharding: 8 cores = 2 batches x 4 head-groups (4 heads each).
Core c handles batch b = c//4 and heads 4*(c%4) .. 4*(c%4)+4 (feature
slice of width F=256). Each core computes its partial output-projection
contribution [S, D] in bf16; the host sums the 4 head-group partials per
batch in f32 and adds b4.

Device dataflow ("transposed world", zero-layout-change matmuls):
  qT = W1g @ x_q.T  [F, S]      kT = W2g @ x_k.T  [F, S]
  vT = W3g @ x_v.T  [F, S]  -> PE-transposed per 128-block into
  v   [S, F] with interleaved ones columns (softmax denominator trick)
  scoresT = kT_h.T @ qT_h  [S_keys, 512q x 2 heads packed]  (K=64; the two
            head MMs are a row-tiled concurrent pair, tile_position (0,0)/(64,0))
  attnT = exp(scoresT / 8)  one ACT instr per key tile, FD=1024
  pv = [v_h | ones].T @ attnT  [65, 512]; row 64 = denominator
  outT_h = pv[0:64] * recip(pv[64])  (reciprocal_approx_fast + gpsimd bcast)
  partial = outT.T @ W4g.T  [S, D] interleaved into later windows as PE filler

Schedule: the 8 attention windows (2 head-pairs x 4 query-quarters) run as
ONE flat software-pipelined stream over 128 (window, key-tile) steps:
scores(j+1) is emitted before pv(j), so the PE keeps streaming across
window boundaries and the ScalarE exp pipe (the ~140us roofline engine)
never gaps. DMA order xk | xv | xq so each projection starts as its
inputs land. PSUM: sc 2x2 banks + pv 2x1 + w4 2x1 = 8 banks exactly.
"""

import numpy as np
import ml_dtypes

import concourse.bass as bass
import concourse.mybir as mybir
import concourse.tile as tile
from concourse import bacc, dve_ops
from concourse.bass_utils import run_bass_kernel_spmd
from concourse.dve_spec import C0, C1, C2, Spec, Src0, lower
from concourse.dve_uop import DveOpSpec

# --- custom DVE exp: w = 2^(s/8 - 6) via quadratic-poly + 4 squarings -------
# Softmax is scale-invariant, so the fixed 2^-6 offset (folded into the
# coefficients, matched by bias=-6ln2 on the ScalarE exps) cancels in the
# denominator. Coefficients minimax-fit on s~N(0,64) for softmax-PV error
# (~0.14% output rel err); u stays in (0.5, 1.1) for |s|<=50, so the four
# squarings are stable. This lets the DVE carry part of the exp stream that
# otherwise hard-bounds the kernel at ScalarE's ~1 elem/cycle.
# coefficients fit on the REAL score distribution (std ~11.4, not the naive
# N(0,64) model) by minimizing softmax-PV output error; held-out ~0.6%.
# u = c1 + s*c0 + s^2*c2 stays positive for all |s| <= 98.
_EXPC0 = 0.005817336180379216                     # s-coefficient
_EXPC1 = 0.764347528481592                        # constant term
_EXPC2 = 2.9445348909278812e-05                   # s^2-coefficient
_EXP_BIAS = -6.0 * 0.6931471805599453             # ScalarE exp bias (=-6ln2)


def _register_exp2_op():
    name = "EXP2_SQ16_ANT"
    for op in dve_ops.OPS:
        if op.name == name:
            return op
    u = (Src0 * C2 + C0) * Src0 + C1
    for _ in range(4):
        u = u * u

    def _ref(in0, in1, c0, c1, c2):
        s = in0.astype(np.float32)
        u = ((s * np.float32(c2) + np.float32(c0)) * s + np.float32(c1)).astype(
            np.float32)
        for _ in range(4):
            u = (u * u).astype(np.float32)
        return u

    op = dve_ops.DveOp(name, Spec(body=u, reference=_ref), subdim=False,
                       uops_sha={})
    opcode = max(dve_ops._SUB_OPCODE_FOR_NAME.values()) + 1
    assert opcode < 0x20
    dve_ops._SUB_OPCODE_FOR_NAME[name] = opcode
    for ver in ("v3", "v4"):
        spec = DveOpSpec(name=name, opcode=opcode, uops=lower(op.spec, ver=ver),
                         rd1_en=False)
        op.uops_sha[ver] = spec.sha(ver)
    dve_ops.OPS.append(op)
    dve_ops.CUSTOM_DVE_SPECS[name] = op.spec
    return op


_EXP2_OP = _register_exp2_op()

BF16 = ml_dtypes.bfloat16
F32 = mybir.dt.float32
BF = mybir.dt.bfloat16

B, S, D = 2, 2048, 1024
H_CORE = 4          # heads per core
DK = 64             # head dim
F = H_CORE * DK     # features per core = 256
P = 128             # partitions
KB = D // P         # k blocks in D contraction = 8
SM = S // P         # seq tiles of 128 = 16
QW = 512            # query window width (per head)
NQW = S // QW       # query quarters = 4
N_CORES = 8
VW = H_CORE * (DK + 1)  # v with interleaved ones columns = 260


def _build_kernel():
    nc = bacc.Bacc(
        "TRN2",
        target_bir_lowering=False,
        debug=False,
        enable_asserts=False,
        num_devices=N_CORES,
    )

    xq = nc.dram_tensor("xq_t", [D, S], BF, kind="ExternalInput").ap()
    xk = nc.dram_tensor("xk_t", [D, S], BF, kind="ExternalInput").ap()
    xv = nc.dram_tensor("xv_t", [D, S], BF, kind="ExternalInput").ap()
    w1 = nc.dram_tensor("w1t", [D, F], BF, kind="ExternalInput").ap()
    w2 = nc.dram_tensor("w2t", [D, F], BF, kind="ExternalInput").ap()
    w3 = nc.dram_tensor("w3t", [D, F], BF, kind="ExternalInput").ap()
    w4 = nc.dram_tensor("w4t", [F, D], BF, kind="ExternalInput").ap()
    b1 = nc.dram_tensor("b1c", [P, F // P], F32, kind="ExternalInput").ap()
    b2 = nc.dram_tensor("b2c", [P, F // P], F32, kind="ExternalInput").ap()
    b3 = nc.dram_tensor("b3c", [P, F // P], F32, kind="ExternalInput").ap()
    ident = nc.dram_tensor("ident", [P, P], BF, kind="ExternalInput").ap()
    out = nc.dram_tensor("out", [S, D], BF, kind="ExternalOutput").ap()

    with tile.TileContext(nc) as tc:
        _body(tc, xq, xk, xv, w1, w2, w3, w4, b1, b2, b3, ident, out)

    nc.compile()
    return nc


def _body(tc, xq, xk, xv, w1, w2, w3, w4, b1, b2, b3, ident, out):
    nc = tc.nc
    MF = F // P  # head-pair tiles in the F=256 feature dim = 2
    EXP = mybir.ActivationFunctionType.Exp

    with (
        tc.tile_pool(name="wpool", bufs=1) as wpool,
        tc.tile_pool(name="xt", bufs=1) as xt_pool,
        tc.tile_pool(name="persist", bufs=1) as persist,
        tc.tile_pool(name="attn", bufs=8) as attn_pool,
        tc.tile_pool(name="small", bufs=4) as small,
        tc.tile_pool(name="stage", bufs=2) as stage,
        tc.tile_pool(name="psum", bufs=1, space="PSUM") as psum,
    ):
        # ---- weight / constant holder tiles ----
        w1_sb = [wpool.tile([P, F], BF, name=f"w1_{k}", tag=f"w1_{k}") for k in range(KB)]
        w2_sb = [wpool.tile([P, F], BF, name=f"w2_{k}", tag=f"w2_{k}") for k in range(KB)]
        w3_sb = [wpool.tile([P, F], BF, name=f"w3_{k}", tag=f"w3_{k}") for k in range(KB)]
        w4_sb = [wpool.tile([P, D], BF, name=f"w4_{k}", tag=f"w4_{k}") for k in range(MF)]
        b1_sb = wpool.tile([P, MF], F32, name="b1_sb", tag="b1_sb")
        b2_sb = wpool.tile([P, MF], F32, name="b2_sb", tag="b2_sb")
        b3_sb = wpool.tile([P, MF], F32, name="b3_sb", tag="b3_sb")
        id_sb = wpool.tile([P, P], BF, name="id_sb", tag="id_sb")
        expb = wpool.tile([P, 1], F32, name="expb", tag="expb")
        nc.vector.memset(expb[:], _EXP_BIAS)

        # persistent activations
        qT = [persist.tile([P, S], BF, name=f"qT_{m}", tag=f"qT_{m}") for m in range(MF)]
        kT = [persist.tile([P, S], BF, name=f"kT_{m}", tag=f"kT_{m}") for m in range(MF)]
        vT = [persist.tile([P, S], BF, name=f"vT_{m}", tag=f"vT_{m}") for m in range(MF)]
        v_sb = [persist.tile([P, VW], BF, name=f"v_{s}", tag=f"v_{s}") for s in range(SM)]
        for s in range(SM):
            for h in range(H_CORE):
                nc.vector.memset(v_sb[s][:, h * (DK + 1) + DK: h * (DK + 1) + DK + 1], 1.0)
        outT = [persist.tile([P, S], BF, name=f"outT_{m}", tag=f"outT_{m}") for m in range(MF)]

        # ---- DMA order: k inputs, then v, then q, so each projection
        #      starts as soon as its tiles land ----
        def dma_w(w_sb_list, w_dram, nk):
            for k in range(nk):
                nc.sync.dma_start(w_sb_list[k][:], w_dram[k * P:(k + 1) * P, :])

        def dma_x(name, x_dram):
            ts = []
            for k in range(KB):
                t = xt_pool.tile([P, S], BF, name=f"x{name}_{k}", tag=f"xt_{name}_{k}",
                                 bufs=1)
                nc.sync.dma_start(t[:], x_dram[k * P:(k + 1) * P, :])
                ts.append(t)
            return ts

        # first-MM inputs first: w2 tile 0 + xk tile 0 land ~5us before the
        # rest, so the PE starts as early as the DMA startup allows
        nc.sync.dma_start(w2_sb[0][:], w2[0:P, :])
        xk_sb = []
        t0 = xt_pool.tile([P, S], BF, name="xk_0", tag="xt_k_0", bufs=1)
        nc.sync.dma_start(t0[:], xk[0:P, :])
        xk_sb.append(t0)
        for k in range(1, KB):
            nc.sync.dma_start(w2_sb[k][:], w2[k * P:(k + 1) * P, :])
        nc.sync.dma_start(b2_sb[:], b2[:])
        for k in range(1, KB):
            t = xt_pool.tile([P, S], BF, name=f"xk_{k}", tag=f"xt_k_{k}", bufs=1)
            nc.sync.dma_start(t[:], xk[k * P:(k + 1) * P, :])
            xk_sb.append(t)
        dma_w(w3_sb, w3, KB)
        nc.sync.dma_start(b3_sb[:], b3[:])
        nc.sync.dma_start(id_sb[:], ident[:])
        xv_sb = dma_x("v", xv)
        dma_w(w1_sb, w1, KB)
        nc.sync.dma_start(b1_sb[:], b1[:])
        # xq split per tile: quarter-0 columns first (all that q-m0q0 and
        # window 0 consume), so the stream isn't gated on the full 4MB of xq
        xq_sb = []
        for k in range(KB):
            t = xt_pool.tile([P, S], BF, name=f"xq_{k}", tag=f"xt_q_{k}", bufs=1)
            nc.sync.dma_start(t[:, 0:512], xq[k * P:(k + 1) * P, 0:512])
            xq_sb.append(t)
        for k in range(KB):
            nc.sync.dma_start(xq_sb[k][:, 512:S], xq[k * P:(k + 1) * P, 512:S])
        dma_w(w4_sb, w4, MF)

        # ---- projections to transposed layout [F-slice, S] ----
        # The 4 query-quarter MMs per (m, k) share the same stationary weights;
        # skip the redundant LDWEIGHTS on the last 3. Bias rides on ScalarE
        # (idle during projections) instead of the DVE.
        IDENT = mybir.ActivationFunctionType.Identity

        # Both head-pairs' matmuls run per input tile (8 MMs/tile), so the PE
        # outpaces the ~1.5us/tile DMA and the k-loop never stalls on input.
        # m0's four quarter-accumulators live in the sc tag, m1's in the
        # pv + w4 tags (idle during projections) = all 8 PSUM banks.
        def proj_qk(x_sb, w_sb, b_sb, dst):
            ps0 = [psum.tile([P, 1024], F32, name=f"pp0_{i}", tag="sc", bufs=2)
                   for i in range(2)]
            ps1 = [psum.tile([P, 512], F32, name=f"pp1a_{i}", tag="pv", bufs=2)
                   for i in range(2)]
            ps1 += [psum.tile([P, 512], F32, name=f"pp1b_{i}", tag="w4", bufs=2)
                    for i in range(2)]
            for k in range(KB):
                for qq in range(4):
                    nc.tensor.matmul(
                        ps0[qq // 2][:, (qq % 2) * 512:(qq % 2 + 1) * 512],
                        w_sb[k][:, 0:P],
                        x_sb[k][:, qq * 512:(qq + 1) * 512],
                        start=(k == 0),
                        stop=(k == KB - 1),
                    )
                for qq in range(4):
                    nc.tensor.matmul(
                        ps1[qq][:],
                        w_sb[k][:, P:2 * P],
                        x_sb[k][:, qq * 512:(qq + 1) * 512],
                        start=(k == 0),
                        stop=(k == KB - 1),
                    )
            for i in range(2):
                nc.scalar.activation(
                    dst[0][:, i * 1024:(i + 1) * 1024], ps0[i][:], IDENT,
                    bias=b_sb[:, 0:1],
                )
            for qq in range(4):
                nc.scalar.activation(
                    dst[1][:, qq * 512:(qq + 1) * 512], ps1[qq][:], IDENT,
                    bias=b_sb[:, 1:2],
                )

        proj_qk(xk_sb, w2_sb, b2_sb, kT)
        proj_qk(xv_sb, w3_sb, b3_sb, vT)

        # vT -> v: PE transpose each [128,128] block, then cast the two
        # 64-wide head slices into v_sb around the ones columns.
        for s in range(SM):
            for m in range(MF):
                tp = psum.tile([P, P], BF, name=f"tp_{s}_{m}", tag="pv", bufs=2)
                nc.tensor.transpose(tp[:], vT[m][:, s * P:(s + 1) * P], id_sb[:])
                for hh in range(2):
                    h = m * 2 + hh
                    nc.vector.tensor_copy(
                        v_sb[s][:, h * (DK + 1): h * (DK + 1) + DK],
                        tp[:, hh * DK:(hh + 1) * DK],
                    )

        # q-proj: only (head-pair 0, quarter 0) is needed at stream start.
        # Every other (m, quarter) group streams into the windows as PE
        # pre-fill in deadline order — window (hp, qw) first reads qT[hp]
        # quarter qw at stream step (2*qw + hp)*16, and each group finishes
        # ~9 steps after it starts draining.
        qps = psum.tile([P, 512], F32, name="qps0", tag="w4", bufs=2)
        for k in range(KB):
            nc.tensor.matmul(qps[:], w1_sb[k][:, 0:P], xq_sb[k][:, 0:512],
                             start=(k == 0), stop=(k == KB - 1))
        nc.scalar.activation(qT[0][:, 0:512], qps[:], IDENT, bias=b1_sb[:, 0:1])

        def gen_qpre():
            for m, qq in ((1, 0), (0, 1), (1, 1), (0, 2), (1, 2), (0, 3), (1, 3)):
                ps = psum.tile([P, 512], F32, name=f"qpre_{m}_{qq}", tag="w4",
                               bufs=2)
                for k in range(KB):
                    nc.tensor.matmul(
                        ps[:],
                        w1_sb[k][:, m * P:(m + 1) * P],
                        xq_sb[k][:, qq * 512:(qq + 1) * 512],
                        start=(k == 0),
                        stop=(k == KB - 1),
                    )
                    yield
                nc.vector.tensor_scalar_add(
                    qT[m][:, qq * 512:(qq + 1) * 512], ps[:], b1_sb[:, m:m + 1]
                )
                yield

        # ---- output projection generator (PE filler inside windows).
        #      tail=True routes the PSUM->SBUF copies to ScalarE (idle once
        #      the exps are done) to shorten the final-quarter tail. ----
        def gen_w4(qts, tail=False):
            for qt in qts:
                ob = stage.tile([P, D], BF, name=f"ob_{qt}", tag="ob", bufs=2)
                for oc in range(2):
                    ps = psum.tile([P, 512], F32, name=f"po_{qt}_{oc}", tag="w4", bufs=2)
                    for m in range(MF):
                        nc.tensor.matmul(
                            ps[:],
                            outT[m][:, qt * P:(qt + 1) * P],
                            w4_sb[m][:, oc * 512:(oc + 1) * 512],
                            start=(m == 0),
                            stop=(m == MF - 1),
                        )
                    if tail:
                        nc.scalar.copy(ob[:, oc * 512:(oc + 1) * 512], ps[:])
                    else:
                        nc.vector.tensor_copy(ob[:, oc * 512:(oc + 1) * 512], ps[:])
                    yield
                nc.sync.dma_start(out[qt * P:(qt + 1) * P, :], ob[:])
                yield

        # ---- attention: one flat pipelined stream over all 8 windows ----
        ESPL = 640  # exp column split: ScalarE does [0:640], DVE [640:1024]
        wins = [(hp, qw) for qw in range(NQW) for hp in range(MF)]
        NW = len(wins)
        pv_ps = {}      # (w_i, h2) -> psum tile
        attn_t = {}     # j -> attn tile

        def emit_scores(j):
            w_i, kt = divmod(j, SM)
            hp, qw = wins[w_i]
            sc = psum.tile([P, 1024], F32, name=f"sc_{j}", tag="sc", bufs=2)
            for h2 in range(2):
                rsl = slice(h2 * DK, (h2 + 1) * DK)
                nc.tensor.matmul(
                    sc[:, h2 * 512:(h2 + 1) * 512],
                    kT[hp][rsl, kt * P:(kt + 1) * P],
                    qT[hp][rsl, qw * QW:(qw + 1) * QW],
                    start=True,
                    stop=True,
                )
            at = attn_pool.tile([P, 1024], BF, name=f"at_{j}", tag="attnT", bufs=8)
            nc.scalar.activation(at[:], sc[:], EXP,
                                 scale=1.0 / np.sqrt(DK), bias=expb[:])
            attn_t[j] = at

        def emit_pv(j):
            w_i, kt = divmod(j, SM)
            hp, qw = wins[w_i]
            if kt == 0:
                for h2 in range(2):
                    pv_ps[(w_i, h2)] = psum.tile(
                        [DK + 1, QW], F32, name=f"pvps_{w_i}_{h2}", tag="pv", bufs=2)
            for h2 in range(2):
                h = hp * 2 + h2
                vsl = slice(h * (DK + 1), h * (DK + 1) + DK + 1)
                nc.tensor.matmul(
                    pv_ps[(w_i, h2)][:],
                    v_sb[kt][:, vsl],
                    attn_t[j][:, h2 * 512:(h2 + 1) * 512],
                    start=(kt == 0),
                    stop=(kt == SM - 1),
                )
            del attn_t[j]

        def emit_norm(w_i, tail=False):
            hp, qw = wins[w_i]
            qsl = slice(qw * QW, (qw + 1) * QW)
            # PSUM pulls split across ScalarE (h0) and DVE (h1) so neither
            # exp queue eats the whole boundary cost; recip stays on DVE,
            # broadcast + the outT scale run on the otherwise-idle GPSIMD.
            dens, raws = [], []
            for h2 in range(2):
                pv = pv_ps.pop((w_i, h2))
                den = small.tile([1, QW], F32, name=f"den_{w_i}_{h2}", tag="den", bufs=3)
                raw = small.tile([DK, QW], BF, name=f"raw_{w_i}_{h2}", tag="raw", bufs=3)
                if tail:
                    nc.scalar.copy(den[:], pv[DK:DK + 1, :])
                    nc.scalar.copy(raw[:], pv[0:DK, :])
                else:
                    nc.vector.tensor_copy(den[:], pv[DK:DK + 1, :])
                    nc.vector.tensor_copy(raw[:], pv[0:DK, :])
                dens.append(den)
                raws.append(raw)
            for h2 in range(2):
                rec = small.tile([1, QW], F32, name=f"rec_{w_i}_{h2}", tag="rec", bufs=3)
                nc.vector.reciprocal_approx_fast(rec[:], dens[h2][:])
                rb = small.tile([1, QW], BF, name=f"rb_{w_i}_{h2}", tag="rb", bufs=3)
                nc.vector.tensor_copy(rb[:], rec[:])
                bc = small.tile([DK, QW], BF, name=f"bc_{w_i}_{h2}", tag="bc", bufs=2)
                nc.gpsimd.partition_broadcast(bc[:], rb[:])
                nc.vector.tensor_mul(
                    outT[hp][h2 * DK:(h2 + 1) * DK, qsl], raws[h2][:], bc[:]
                )

        NSTEP = NW * SM
        fill = iter(())
        prefill = gen_qpre()
        norm_due = None
        emit_scores(0)
        for j in range(1, NSTEP):
            emit_scores(j)
            emit_pv(j - 1)
            next(prefill, None)
            w_prev, kt_prev = divmod(j - 1, SM)
            if kt_prev == SM - 1:
                norm_due = w_prev
                hp_p, qw_p = wins[w_prev]
                if hp_p == MF - 1:  # quarter qw_p fully done -> queue its W4
                    fill = gen_w4(range(qw_p * 4, (qw_p + 1) * 4))
            # defer the normalize a few steps so its DVE ops don't
            # head-of-line block the new window's start
            if j % SM == 4 and norm_due is not None:
                emit_norm(norm_due)
                norm_due = None
            # consume filler only late in the window: its first MMs wait on
            # the previous window's normalize muls, which would head-of-line
            # block the PE queue earlier.
            if j % SM >= 9:
                next(fill, None)
        emit_pv(NSTEP - 1)
        if norm_due is not None:
            emit_norm(norm_due)
        for _ in fill:
            pass
        # last window: normalize per query-half (halves align with W4 output
        # qtiles), so W4 for qts 12/13 runs while the second half is still
        # normalizing on the Scalar/DVE/GpSimd queues.
        hp_l, qw_l = wins[NW - 1]
        HQ = QW // 2
        pvs = [pv_ps.pop((NW - 1, h2)) for h2 in range(2)]
        for half in range(2):
            cs = slice(half * HQ, (half + 1) * HQ)
            qsl = slice(qw_l * QW + half * HQ, qw_l * QW + (half + 1) * HQ)
            for h2 in range(2):
                den = small.tile([1, HQ], F32, name=f"tden_{half}_{h2}", tag="den", bufs=3)
                raw = small.tile([DK, HQ], BF, name=f"traw_{half}_{h2}", tag="raw", bufs=3)
                nc.scalar.copy(den[:], pvs[h2][DK:DK + 1, cs])
                nc.scalar.copy(raw[:], pvs[h2][0:DK, cs])
                rec = small.tile([1, HQ], F32, name=f"trec_{half}_{h2}", tag="rec", bufs=3)
                nc.vector.reciprocal_approx_fast(rec[:], den[:])
                rb = small.tile([1, HQ], BF, name=f"trb_{half}_{h2}", tag="rb", bufs=3)
                nc.vector.tensor_copy(rb[:], rec[:])
                bc = small.tile([DK, HQ], BF, name=f"tbc_{half}_{h2}", tag="bc", bufs=2)
                nc.gpsimd.partition_broadcast(bc[:], rb[:])
                nc.vector.tensor_mul(
                    outT[hp_l][h2 * DK:(h2 + 1) * DK, qsl], raw[:], bc[:]
                )
            base = (NQW - 1) * 4 + half * 2
            for _ in gen_w4(range(base, base + 2), tail=True):
                pass


_NC_CACHE = None


def _get_nc():
    global _NC_CACHE
    if _NC_CACHE is None:
        _NC_CACHE = _build_kernel()
    return _NC_CACHE


def _make_in_maps(query, key, value, W1, b1, W2, b2, W3, b3, W4, b4):
    in_maps = []
    ident = np.eye(P, dtype=BF16)
    for c in range(N_CORES):
        b, g = divmod(c, 4)
        gs = slice(g * F, (g + 1) * F)
        in_maps.append({
            "xq_t": np.ascontiguousarray(query[b].T).astype(BF16),
            "xk_t": np.ascontiguousarray(key[b].T).astype(BF16),
            "xv_t": np.ascontiguousarray(value[b].T).astype(BF16),
            "w1t": np.ascontiguousarray(W1[gs, :].T).astype(BF16),
            "w2t": np.ascontiguousarray(W2[gs, :].T).astype(BF16),
            "w3t": np.ascontiguousarray(W3[gs, :].T).astype(BF16),
            "w4t": np.ascontiguousarray(W4[:, gs].T).astype(BF16),
            "b1c": np.ascontiguousarray(b1[gs].reshape(F // P, P).T).astype(np.float32),
            "b2c": np.ascontiguousarray(b2[gs].reshape(F // P, P).T).astype(np.float32),
            "b3c": np.ascontiguousarray(b3[gs].reshape(F // P, P).T).astype(np.float32),
            "ident": ident,
        })
    return in_maps


def kernel(query, key, value, W1, b1, W2, b2, W3, b3, W4, b4, _trace=False, _tmpdir=None):
    args = [np.asarray(a) for a in (query, key, value, W1, b1, W2, b2, W3, b3, W4, b4)]
    nc = _get_nc()
    in_maps = _make_in_maps(*args)
    res = run_bass_kernel_spmd(
        nc, in_maps, core_ids=list(range(N_CORES)),
        trace=_trace, tmpdir=_tmpdir,
    )
    b4_f = args[10].astype(np.float32)
    full = np.zeros((B, S, D), np.float32)
    for c in range(N_CORES):
        full[c // 4] += res.results[c]["out"].astype(np.float32)
    full += b4_f[None, None, :]
    kernel.last_results = res
    return full
